# revision 2
# baseline (speedup 1.0000x reference)
"""LocationAwareAttention Trainium2 kernel (8 NeuronCores, data-parallel over batch).

Key algebraic restructure: the huge psi projection (B,U,D)x(D,H*hd) is never
materialized.  energy[b,h,u] = enc[b,u,:] . w_b[:,h] + c[b,h] + loc_bias[b,h,u]
with w_b = Wpsi[h] @ phi_s[b,h]  (phi_s = dec @ Wphi + bphi), c = phi_s . bpsi.
loc_bias folds conv_kernel and Wloc into one (31,H) kernel Keff applied via a
shifted-window matmul.  The kernel is then purely memory-bound on streaming
enc_state once (f32 -> fp16 cast inside the load DMA), with on-chip PE
transposes providing encT for the energy matmul and resident fp16 enc feeding
the context matmul.
"""

import sys

sys.path.insert(0, "/opt/trn_rl_repo")

from contextlib import ExitStack

import numpy as np

import concourse.bass as bass
import concourse.tile as tile
from concourse import bacc, mybir
from concourse.bass_utils import run_bass_kernel_spmd

B, U, D = 32, 4096, 512
H, HD = 4, 128
CK, PAD = 31, 15
NC = 8
BL = B // NC  # 4 batches per core
UP = U + 2 * PAD  # 4126
NT = U // 128  # 32 u-tiles per batch
F32 = mybir.dt.float32
F16 = mybir.dt.float16

_CACHE = {}


def _build():
    nc = bacc.Bacc(
        "TRN2",
        target_bir_lowering=False,
        debug=False,
        enable_asserts=False,
        num_devices=NC,
    )
    enc_d = nc.dram_tensor("enc", (BL, U, D), F32, kind="ExternalInput").ap()
    prevp_d = nc.dram_tensor("prevp", (BL, UP), F32, kind="ExternalInput").ap()
    decT_d = nc.dram_tensor("decT", (D, BL), F32, kind="ExternalInput").ap()
    wphi_d = nc.dram_tensor("wphi", (D, H * HD), F32, kind="ExternalInput").ap()
    wpsiT_d = nc.dram_tensor("wpsiT", (HD, H * D), F32, kind="ExternalInput").ap()
    keff_d = nc.dram_tensor("keff", (CK, H), F32, kind="ExternalInput").ap()
    bphi_d = nc.dram_tensor("bphi_r", (BL, H * HD), F32, kind="ExternalInput").ap()
    bpsi_d = nc.dram_tensor("bpsi_r", (BL, H * HD), F32, kind="ExternalInput").ap()
    bpsiT_d = nc.dram_tensor("bpsiT", (HD, H), F32, kind="ExternalInput").ap()
    wout_d = nc.dram_tensor("wout", (H * D, 512), F32, kind="ExternalInput").ap()
    bout_d = nc.dram_tensor("bout_r", (BL, 512), F32, kind="ExternalInput").ap()
    iden16_d = nc.dram_tensor("iden16", (128, 128), F16, kind="ExternalInput").ap()
    iden32_d = nc.dram_tensor("iden32", (128, 128), F32, kind="ExternalInput").ap()
    ones32_d = nc.dram_tensor("ones32", (1, 128), F32, kind="ExternalInput").ap()
    cv_d = nc.dram_tensor("cv", (BL, 512), F32, kind="ExternalOutput").ap()
    aw_d = nc.dram_tensor("aw", (BL, U), F32, kind="ExternalOutput").ap()

    with tile.TileContext(nc) as tc, ExitStack() as ctx:
        const = ctx.enter_context(tc.tile_pool(name="const", bufs=1))
        encp = ctx.enter_context(tc.tile_pool(name="encp", bufs=2))
        work = ctx.enter_context(tc.tile_pool(name="work", bufs=3))
        sm = ctx.enter_context(tc.tile_pool(name="sm", bufs=2))
        ptr = ctx.enter_context(tc.tile_pool(name="ptr", bufs=2, space="PSUM"))
        pe_ = ctx.enter_context(tc.tile_pool(name="pe", bufs=2, space="PSUM"))
        pcx = ctx.enter_context(tc.tile_pool(name="pcx", bufs=2, space="PSUM"))
        pmi = ctx.enter_context(tc.tile_pool(name="pmi", bufs=2, space="PSUM"))

        # ---------------- prelude: params ----------------
        iden16 = const.tile([128, 128], F16)
        nc.sync.dma_start(iden16[:], iden16_d[:])
        iden32 = const.tile([128, 128], F32)
        nc.sync.dma_start(iden32[:], iden32_d[:])
        ones32 = const.tile([1, 128], F32)
        nc.sync.dma_start(ones32[:], ones32_d[:])
        decT = const.tile([128, 4 * BL], F32)  # [p, c*BL+b]
        nc.sync.dma_start(decT[:], decT_d.rearrange("(c p) b -> p c b", p=128))
        wphi = const.tile([128, 4 * H * HD], F32)  # [p, c*512 + h*128 + k]
        nc.sync.dma_start(wphi[:], wphi_d.rearrange("(c p) f -> p c f", p=128))
        wpsiT = const.tile([128, H * D], F32)  # [k, h*512 + d]
        nc.sync.dma_start(wpsiT[:], wpsiT_d[:])
        keff16 = const.tile([CK, H], F16)
        nc.gpsimd.dma_start(keff16[:], keff_d[:])
        bphi = const.tile([BL, H * HD], F32)
        nc.sync.dma_start(bphi[:], bphi_d[:])
        bpsi = const.tile([BL, H * HD], F32)
        nc.sync.dma_start(bpsi[:], bpsi_d[:])
        bpsiT = const.tile([HD, H], F32)
        nc.sync.dma_start(bpsiT[:], bpsiT_d[:])
        bout = const.tile([BL, 512], F32)
        nc.sync.dma_start(bout[:], bout_d[:])
        wout16 = const.tile([128, 16 * 512], F16)  # [p, c*512+n]
        nc.gpsimd.dma_start(wout16[:], wout_d.rearrange("(c p) n -> p c n", p=128))
        P16 = const.tile([CK, BL * U], F16)  # shifted prev windows, [k, b*U+u]
        for b in range(BL):
            src = bass.AP(prevp_d.tensor, b * UP, [[1, CK], [1, U]])
            nc.gpsimd.dma_start(P16[:, b * U : (b + 1) * U], src)

        # ---------------- prelude: phi, c, w ----------------
        phi32 = const.tile([BL, H * HD], F32)  # [b, h*128+k]
        phiT = const.tile([128, H * BL], F32)  # [k, h*BL+b]
        for h in range(H):
            pphi = pmi.tile([BL, HD], F32, tag="pmi")
            for dc in range(4):
                nc.tensor.matmul(
                    pphi[:],
                    lhsT=decT[:, dc * BL : (dc + 1) * BL],
                    rhs=wphi[:, dc * 512 + h * HD : dc * 512 + (h + 1) * HD],
                    start=(dc == 0),
                    stop=(dc == 3),
                )
            nc.vector.tensor_add(
                phi32[:, h * HD : (h + 1) * HD], pphi[:], bphi[:, h * HD : (h + 1) * HD]
            )
            pT = pmi.tile([HD, BL], F32, tag="pmi")
            nc.tensor.transpose(pT[:], phi32[:, h * HD : (h + 1) * HD], iden32[:BL, :BL])
            nc.vector.tensor_copy(phiT[:, h * BL : (h + 1) * BL], pT[:])

        # c row: c[b,h] = phi[b,h,:] . bpsi[h,:]  ->  (1, h*BL+b) on partition 0
        crow = const.tile([1, H * BL], F32)
        pc = pmi.tile([1, H * BL], F32, tag="pmi")
        for h in range(H):
            nc.tensor.matmul(
                pc[:, h * BL : (h + 1) * BL],
                lhsT=bpsiT[:, h : h + 1],
                rhs=phiT[:, h * BL : (h + 1) * BL],
                start=(h == 0),
                stop=(h == H - 1),
            )
        nc.vector.tensor_copy(crow[:], pc[:])
        # broadcast c to all partitions: (128, h*BL+b)
        pcb = pmi.tile([128, H * BL], F32, tag="pmi")
        nc.tensor.matmul(pcb[:], lhsT=ones32[:], rhs=crow[:], start=True, stop=True)
        # cbrep[p, b*16 + tt*4 + h] = c[b,h]
        cbrep = const.tile([128, BL * 16], F32)
        pcb_v = pcb[:].rearrange("p (h b) -> p b h", h=H)
        for b in range(BL):
            for tt in range(4):
                nc.vector.tensor_copy(
                    cbrep[:, b * 16 + tt * 4 : b * 16 + tt * 4 + 4],
                    pcb_v[:, b : b + 1, :],
                )

        # w16[p, h*16 + j*4 + b] = w_b[j*128+p, h]
        w16 = const.tile([128, H * 4 * BL], F16)
        for h in range(H):
            for j in range(4):
                pw = pmi.tile([HD, BL], F32, tag="pmi")
                nc.tensor.matmul(
                    pw[:],
                    lhsT=wpsiT[:, h * D + j * 128 : h * D + (j + 1) * 128],
                    rhs=phiT[:, h * BL : (h + 1) * BL],
                    start=True,
                    stop=True,
                )
                nc.vector.tensor_copy(
                    w16[:, (h * 4 + j) * BL : (h * 4 + j + 1) * BL], pw[:]
                )
        w16_v = w16[:].rearrange("p (h f) -> p f h", h=H)  # (p, j*4+b, h)

        comb = const.tile([128, BL * 16], F16)  # [p, b*16 + s*4 + h]

        # ---------------- main loop over local batches ----------------
        for b in range(BL):
            enc16 = encp.tile([128, NT * 512], F16, tag="enc")  # [ui, t*512 + d]
            for half in range(2):
                src = bass.AP(
                    enc_d.tensor,
                    b * U * D + half * 16 * 128 * D,
                    [[D, 128], [128 * D, 16], [1, D]],
                )
                nc.gpsimd.dma_start(
                    enc16[:, half * 16 * 512 : (half + 1) * 16 * 512], src
                )

            e_b = sm.tile([128, NT * H], F32, tag="eb")  # [ui, t*4+h]
            pe_t = None
            for t in range(NT):
                ptr_t = ptr.tile([128, 512], F16, tag="ptr")
                for j in range(4):
                    nc.tensor.transpose(
                        ptr_t[:, j * 128 : (j + 1) * 128],
                        enc16[:, t * 512 + j * 128 : t * 512 + (j + 1) * 128],
                        iden16[:],
                    )
                encT = work.tile([128, 512], F16, tag="encT")
                if t % 2 == 0:
                    nc.vector.tensor_copy(encT[:], ptr_t[:])
                else:
                    nc.scalar.copy(encT[:], ptr_t[:])
                tt = t % 4
                if tt == 0:
                    pe_t = pe_.tile([128, 16], F32, tag="pe")
                nc.tensor.matmul(
                    pe_t[:, tt * 4 : tt * 4 + 4],
                    lhsT=P16[:, b * U + t * 128 : b * U + (t + 1) * 128],
                    rhs=keff16[:],
                    start=True,
                    stop=False,
                )
                for j in range(4):
                    nc.tensor.matmul(
                        pe_t[:, tt * 4 : tt * 4 + 4],
                        lhsT=encT[:, j * 128 : (j + 1) * 128],
                        rhs=w16_v[:, j * 4 + b : j * 4 + b + 1, :],
                        start=False,
                        stop=(j == 3),
                    )
                if tt == 3:
                    blk = t // 4
                    nc.vector.tensor_add(
                        e_b[:, blk * 16 : (blk + 1) * 16],
                        pe_t[:],
                        cbrep[:, b * 16 : (b + 1) * 16],
                    )

            # ---------------- softmax over u (4096) per head ----------------
            e_v = e_b[:].rearrange("p (t h) -> p h t", h=H)
            mx1 = sm.tile([128, H], F32, tag="mx1")
            nc.vector.reduce_max(mx1[:], e_v, mybir.AxisListType.X)
            pmxT = pmi.tile([H, 128], F32, tag="pmi")
            nc.tensor.transpose(pmxT[:], mx1[:], iden32[:])
            m4 = sm.tile([H, 1], F32, tag="m4")
            nc.vector.reduce_max(m4[:], pmxT[:], mybir.AxisListType.X)
            pm1 = pmi.tile([1, H], F32, tag="pmi")
            nc.tensor.transpose(pm1[:], m4[:], iden32[:H, :H])
            mrow = sm.tile([1, 128], F32, tag="mrow")
            nc.vector.tensor_scalar_mul(mrow[:, 0:4], pm1[:], -1.0)
            for w_, s_ in ((4, 4), (8, 8), (16, 16), (32, 32), (64, 64)):
                nc.vector.tensor_copy(mrow[:, s_ : s_ + w_], mrow[:, 0:w_])
            pmb = pmi.tile([128, 128], F32, tag="pmi")
            nc.tensor.matmul(pmb[:], lhsT=ones32[:], rhs=mrow[:], start=True, stop=True)
            nc.vector.tensor_add(e_b[:], e_b[:], pmb[:])
            p16 = sm.tile([128, NT * H], F16, tag="p16")
            nc.scalar.activation(p16[:], e_b[:], mybir.ActivationFunctionType.Exp)

            p_v = p16[:].rearrange("p (t h) -> p h t", h=H)
            zs1 = sm.tile([128, H], F32, tag="zs1")
            nc.vector.reduce_sum(zs1[:], p_v, mybir.AxisListType.X)
            pzT = pmi.tile([H, 128], F32, tag="pmi")
            nc.tensor.transpose(pzT[:], zs1[:], iden32[:])
            z4 = sm.tile([H, 1], F32, tag="z4")
            nc.vector.reduce_sum(z4[:], pzT[:], mybir.AxisListType.X)
            rz = sm.tile([H, 1], F32, tag="rz")
            nc.vector.reciprocal(rz[:], z4[:])

            # attention-weight output: aw[u] = 0.25 * sum_h p[u,h]/Z[h]
            pq1 = pmi.tile([1, H], F32, tag="pmi")
            nc.tensor.transpose(pq1[:], rz[:], iden32[:H, :H])
            qrow = sm.tile([1, 128], F32, tag="qrow")
            nc.vector.tensor_scalar_mul(qrow[:, 0:4], pq1[:], 0.25)
            for w_, s_ in ((4, 4), (8, 8), (16, 16), (32, 32), (64, 64)):
                nc.vector.tensor_copy(qrow[:, s_ : s_ + w_], qrow[:, 0:w_])
            pqb = pmi.tile([128, 128], F32, tag="pmi")
            nc.tensor.matmul(pqb[:], lhsT=ones32[:], rhs=qrow[:], start=True, stop=True)
            pwq = sm.tile([128, NT * H], F32, tag="pwq")
            nc.vector.tensor_mul(pwq[:], p16[:], pqb[:])
            aw_b = sm.tile([128, NT], F32, tag="awb")
            nc.vector.reduce_sum(
                aw_b[:],
                pwq[:].rearrange("p (t h) -> p t h", h=H),
                mybir.AxisListType.X,
            )
            aw_dst = bass.AP(aw_d.tensor, b * U, [[1, 128], [128, NT]])
            nc.sync.dma_start(aw_dst, aw_b[:])

            # ---------------- context ----------------
            pctx = pcx.tile([H, 512], F32, tag="pcx")
            for t in range(NT):
                nc.tensor.matmul(
                    pctx[:],
                    lhsT=p16[:, t * 4 : (t + 1) * 4],
                    rhs=enc16[:, t * 512 : (t + 1) * 512],
                    start=(t == 0),
                    stop=(t == NT - 1),
                )
            ctx16 = sm.tile([H, 512], F16, tag="ctx16")
            nc.vector.tensor_scalar_mul(ctx16[:], pctx[:], rz[:])
            for s in range(4):
                pcm = pmi.tile([128, H], F16, tag="pmi")
                nc.tensor.transpose(
                    pcm[:], ctx16[:, s * 128 : (s + 1) * 128], iden16[:H, :H]
                )
                nc.vector.tensor_copy(
                    comb[:, b * 16 + s * 4 : b * 16 + (s + 1) * 4], pcm[:]
                )

        # ---------------- output projection (all batches) ----------------
        comb_v = comb[:].rearrange("p (b f) -> p f b", b=BL)  # (p, s*4+h, b)
        pcv = pcx.tile([BL, 512], F32, tag="pcx")
        for c in range(16):
            f0 = (c % 4) * 4 + c // 4  # s*4 + h with s=c%4, h=c//4
            nc.tensor.matmul(
                pcv[:],
                lhsT=comb_v[:, f0 : f0 + 1, :],
                rhs=wout16[:, c * 512 : (c + 1) * 512],
                start=(c == 0),
                stop=(c == 15),
            )
        cv_sb = sm.tile([BL, 512], F32, tag="cvsb")
        nc.vector.tensor_add(cv_sb[:], pcv[:], bout[:])
        nc.sync.dma_start(cv_d[:], cv_sb[:])

    nc.compile()
    return nc


def _get_nc():
    if "nc" not in _CACHE:
        _CACHE["nc"] = _build()
    return _CACHE["nc"]


def _prep_in_maps(
    dec_state,
    enc_state,
    prev_attention_weights,
    Wphi,
    bphi,
    Wpsi,
    bpsi,
    conv_kernel,
    Wloc,
    Wout,
    bout,
):
    f32 = np.float32
    dec = np.asarray(dec_state, f32)
    enc = np.ascontiguousarray(np.asarray(enc_state, f32))
    prev = np.asarray(prev_attention_weights, f32)
    Wphi = np.asarray(Wphi, f32)
    bphi = np.asarray(bphi, f32)
    Wpsi = np.asarray(Wpsi, f32)
    bpsi = np.asarray(bpsi, f32)
    keff = np.einsum(
        "kc,ch->kh", np.asarray(conv_kernel, f32)[:, 0, :], np.asarray(Wloc, f32)
    ).astype(f32)
    prevp = np.pad(prev, ((0, 0), (PAD, PAD))).astype(f32)
    wphi_h = np.ascontiguousarray(Wphi.transpose(1, 0, 2).reshape(D, H * HD))
    wpsiT_h = np.ascontiguousarray(Wpsi.transpose(2, 0, 1).reshape(HD, H * D))
    bphi_r = np.ascontiguousarray(np.broadcast_to(bphi.reshape(1, H * HD), (BL, H * HD)))
    bpsi_r = np.ascontiguousarray(np.broadcast_to(bpsi.reshape(1, H * HD), (BL, H * HD)))
    bpsiT_h = np.ascontiguousarray(bpsi.T)
    bout_r = np.ascontiguousarray(
        np.broadcast_to(np.asarray(bout, f32).reshape(1, 512), (BL, 512))
    )
    wout_h = np.ascontiguousarray(np.asarray(Wout, f32))
    iden16 = np.eye(128, dtype=np.float16)
    iden32 = np.eye(128, dtype=f32)
    ones32 = np.ones((1, 128), f32)
    in_maps = []
    for cix in range(NC):
        sl = slice(cix * BL, (cix + 1) * BL)
        in_maps.append(
            {
                "enc": np.ascontiguousarray(enc[sl]),
                "prevp": np.ascontiguousarray(prevp[sl]),
                "decT": np.ascontiguousarray(dec[sl].T),
                "wphi": wphi_h,
                "wpsiT": wpsiT_h,
                "keff": keff,
                "bphi_r": bphi_r,
                "bpsi_r": bpsi_r,
                "bpsiT": bpsiT_h,
                "wout": wout_h,
                "bout_r": bout_r,
                "iden16": iden16,
                "iden32": iden32,
                "ones32": ones32,
            }
        )
    return in_maps


def kernel(**inputs):
    nc = _get_nc()
    in_maps = _prep_in_maps(**inputs)
    res = run_bass_kernel_spmd(nc, in_maps, core_ids=list(range(NC)))
    cv = np.concatenate([res.results[c]["cv"] for c in range(NC)], axis=0)
    aw = np.concatenate([res.results[c]["aw"] for c in range(NC)], axis=0)
    return cv, aw
